# revision 1
# baseline (speedup 1.0000x reference)
"""Trainium2 Bass kernel for nn_DLUPack (CARAFE-style dynamic upsampling).

Sharding: 8 cores = (batch n in [0,4)) x (output-row-parity s in {0,1});
core (n, s) computes low-res rows hh in [32s, 32s+32) -> all parity-s output rows.

Reference output mapping (its reshape scrambles positions):
  ref[n, c, 2y+i, 2x+j] = sum_k patches[c, hh, ww, k] * kern[hh, ww, k, u]
  with hh = 32s + 16jh + m:  row r = 8m + 2(ww//16) + s, col = 8*(ww%16) + 2u + jh.

Device pipeline per core:
  1. compressor 1x1 conv (PE, bf16) -> cx [64, 38, 66]
  2. offset+mask 3x3 convs (9 accumulated MMs) -> psum [57, .]: off ch 0-7, mask ch 32-56
  3. exp in ACT evac, PE-transpose -> expT [64 w, 36 g, 25 k], softmax via free-dim reduce
  4. offset PE-transpose -> deltas; indicator bilinear weights W9 (DVE)
  5. kernc assembly: 9 broadcast-multiply terms (stride-0 APs) + adds (DVE)
  6. kernc -> bf16; 4 partition-shifted variants via SBUF-SBUF DMA
  7. per pair m: 10 data-prep copies -> 2 local_scatter (GPSIMD) -> banded [128, 5x512]
  8. carafe: 5 accumulated MMs [128,128]x[128,512] per (pair, c-half) -> psum [128,512]
  9. ACT evac -> DMA out 4 contiguous output rows
"""
import sys
import numpy as np

sys.path.insert(0, '/opt/trn_rl_repo')

import ml_dtypes  # noqa: E402
from contextlib import ExitStack  # noqa: E402

import concourse.bass as bass  # noqa: E402
import concourse.tile as tile  # noqa: E402
from concourse import mybir, bacc  # noqa: E402
from concourse.bass_utils import run_bass_kernel_spmd  # noqa: E402

F32 = mybir.dt.float32
BF16 = mybir.dt.float16  # NOTE: fp16 (better mantissa), name kept for brevity
I16 = mybir.dt.int16
AF = mybir.ActivationFunctionType
OP = mybir.AluOpType

N, C, H, W = 4, 256, 64, 64


def _ap(base, off_elems, dims):
    return bass.AP(tensor=base.tensor, offset=base.offset + off_elems, ap=[list(d) for d in dims])


def build_scatter_tables():
    idx1 = -np.ones((128, 100), np.int16)
    idx2 = -np.ones((128, 100), np.int16)
    for p in range(128):
        jh, wpp = p // 64, p % 64
        for b in range(5):
            w = wpp + b - 2
            if not (0 <= w < 64):
                continue
            q, wl = w // 16, w % 16
            for ki in range(5):
                for u in range(4):
                    col = q * 128 + 8 * wl + 2 * u + jh
                    qidx = (b * 5 + ki) * 4 + u
                    if ki < 3:
                        idx1[p, qidx] = ki * 512 + col
                    else:
                        idx2[p, qidx] = (ki - 3) * 512 + col
    return idx1, idx2


def build_program():
    nc = bacc.Bacc(None, target_bir_lowering=False, debug=True)

    xwin = nc.declare_dram_parameter('xwin', [2, 128, 38 * 64], BF16, isOutput=False)
    xT2 = nc.declare_dram_parameter('xT2', [128, 20 * 256], BF16, isOutput=False)
    wc = nc.declare_dram_parameter('wc', [128, 2 * 64], BF16, isOutput=False)
    wk = nc.declare_dram_parameter('wk', [64, 9 * 57], BF16, isOutput=False)
    bco = nc.declare_dram_parameter('bco', [57, 1], F32, isOutput=False)
    bcomp = nc.declare_dram_parameter('bcomp', [64, 1], F32, isOutput=False)
    wvec = nc.declare_dram_parameter('wvec', [64, 1], F32, isOutput=False)
    w63 = nc.declare_dram_parameter('w63', [64, 1], F32, isOutput=False)
    hrow = nc.declare_dram_parameter('hrow', [64, 32], F32, isOutput=False)
    y63 = nc.declare_dram_parameter('y63', [64, 32], F32, isOutput=False)
    ident = nc.declare_dram_parameter('ident', [128, 128], F32, isOutput=False)
    idx1 = nc.declare_dram_parameter('idx1', [128, 100], I16, isOutput=False)
    idx2 = nc.declare_dram_parameter('idx2', [128, 100], I16, isOutput=False)
    zed = nc.declare_dram_parameter('zed', [2, 3600], BF16, isOutput=False)
    outp = nc.declare_dram_parameter('outp', [256, 64 * 128], F32, isOutput=True)

    with tile.TileContext(nc) as tc, ExitStack() as ctx:
        sing = ctx.enter_context(tc.tile_pool(name='sing', bufs=1))
        work = ctx.enter_context(tc.tile_pool(name='work', bufs=1))
        loop = ctx.enter_context(tc.tile_pool(name='loop', bufs=3))
        band = ctx.enter_context(tc.tile_pool(name='band', bufs=4))
        rowp = ctx.enter_context(tc.tile_pool(name='rowp', bufs=4))
        psum = ctx.enter_context(tc.psum_pool(name='ps', bufs=2))
        psc = ctx.enter_context(tc.psum_pool(name='psc', bufs=3))

        def load(shape, dtype, src):
            t = sing.tile(shape, dtype, name=f'ld_{src.tensor.name if hasattr(src, "tensor") else id(src)}')
            nc.sync.dma_start(out=t[:], in_=src[:])
            return t

        xwin_sb = sing.tile([128, 2, 38 * 64], BF16)
        for cg_ in range(2):
            nc.sync.dma_start(out=xwin_sb[:, cg_, :],
                              in_=_ap(xwin[:], cg_ * 128 * 2432, [[2432, 128], [1, 2432]]))
        xT2_sb = load([128, 20 * 256], BF16, xT2)
        wc_sb = load([128, 2, 64], BF16, wc)
        wk_sb = load([64, 9, 57], BF16, wk)
        bco_sb = load([57, 1], F32, bco)
        bcomp_sb = load([64, 1], F32, bcomp)
        wvec_sb = load([64, 1], F32, wvec)
        w63_sb = load([64, 1], F32, w63)
        hrow_sb = load([64, 32], F32, hrow)
        y63_sb = load([64, 32], F32, y63)
        id_sb = load([128, 128], F32, ident)
        idx1_sb = load([128, 100], I16, idx1)
        idx2_sb = load([128, 100], I16, idx2)

        # PE warm-up: keep TensorE busy during input-DMA wait so HAM reaches 8/8
        pw = psc.tile([128, 512], F32, name='pcs_warm', tag='pcs')
        for _ in range(90):
            nc.tensor.matmul(pw[0:64, 0:64], id_sb[:, 0:64], id_sb[:, 0:64], start=True, stop=True)

        # hoisted variant buffers; edge partitions zeroed via tiny DMAs from DRAM zeros
        msm4_p1 = work.tile([64, 36 * 100], BF16)
        msm4_m1 = work.tile([64, 36 * 100], BF16)
        nc.sync.dma_start(out=_ap(msm4_p1[:], 63 * 3600, [[3600, 1], [1, 3600]]),
                          in_=_ap(zed[:], 0, [[3600, 1], [1, 3600]]))
        nc.scalar.dma_start(out=_ap(msm4_m1[:], 0, [[3600, 1], [1, 3600]]),
                            in_=_ap(zed[:], 0, [[3600, 1], [1, 3600]]))
        kbf_sh = {}
        for d in (-2, -1, 1, 2):
            kbf_sh[d] = work.tile([64, 3200], BF16, name=f'kbf{d}')
            eng = nc.sync if d > 0 else nc.scalar
            if d > 0:
                eng.dma_start(out=_ap(kbf_sh[d][:], (64 - d) * 3200, [[3200, d], [1, 3200]]),
                              in_=_ap(zed[:], 0, [[3600, d], [1, 3200]]))
            else:
                eng.dma_start(out=_ap(kbf_sh[d][:], 0, [[3200, -d], [1, 3200]]),
                              in_=_ap(zed[:], 0, [[3600, -d], [1, 3200]]))

        # ---- 1. compressor ----
        cx_sb = work.tile([64, 38, 66], BF16)
        nc.vector.memset(_ap(cx_sb[:], 0, [[38 * 66, 64], [66, 38], [1, 1]]), 0.0)
        nc.vector.memset(_ap(cx_sb[:], 65, [[38 * 66, 64], [66, 38], [1, 1]]), 0.0)
        for grp in range(5):
            g0 = grp * 8
            rows = min(8, 38 - g0)
            nn = rows * 64
            pcs = psum.tile([64, 512], F32)
            for cg in range(2):
                nc.tensor.matmul(pcs[:, :nn], wc_sb[:, cg, :],
                                 xwin_sb[:, cg, g0 * 64:g0 * 64 + nn],
                                 start=(cg == 0), stop=(cg == 1))
            nc.scalar.activation(
                out=_ap(cx_sb[:], g0 * 66 + 1, [[38 * 66, 64], [66, rows], [1, 64]]),
                in_=_ap(pcs[:], 0, [[512, 64], [64, rows], [1, 64]]),
                func=AF.Identity, bias=bcomp_sb[:], scale=1.0)

        # ---- 2. offset+mask convs ----
        expS = work.tile([25, 36, 64], F32)
        offS = work.tile([8, 32, 64], F32)
        for grp in range(6):
            g0 = grp * 6
            nn = 6 * 64
            pcs = psum.tile([57, 384], F32)
            for t in range(9):
                dy, dx = t // 3, t % 3
                rhs = _ap(cx_sb[:], (g0 + dy) * 66 + dx, [[38 * 66, 64], [66, 6], [1, 64]])
                nc.tensor.matmul(pcs[:, :nn], wk_sb[:, t, :], rhs,
                                 start=(t == 0), stop=(t == 8))
            nc.scalar.activation(out=expS[:, g0:g0 + 6, :],
                                 in_=_ap(pcs[:], 32 * 384, [[384, 25], [64, 6], [1, 64]]),
                                 func=AF.Exp, bias=bco_sb[32:57], scale=1.0)
            lo, hi = max(g0, 2), min(g0 + 6, 34)
            if lo < hi:
                nc.vector.tensor_scalar(
                    out=offS[:, lo - 2:hi - 2, :],
                    in0=_ap(pcs[:], (lo - g0) * 64, [[384, 8], [64, hi - lo], [1, 64]]),
                    scalar1=bco_sb[0:8], scalar2=None, op0=OP.add)

        # ---- 3. transpose exp -> expT; softmax (pair-batched transposes) ----
        expT = work.tile([64, 36, 25], F32)
        for half in range(2):
            pcnt = 10 if half == 0 else 8      # g-pairs this bank
            pt = psum.tile([128, 512], F32)
            for i in range(pcnt):
                g = (half * 10 + i) * 2
                # in [25, 128] = rows (g, g+1) -> out [128, 25]
                nc.tensor.transpose(pt[:, i * 25:i * 25 + 25],
                                    _ap(expS[:], g * 64, [[36 * 64, 25], [1, 128]]),
                                    id_sb[0:25, 0:25])
            for sub in range(2):
                nc.scalar.activation(
                    out=_ap(expT[:], (half * 20 + sub) * 25,
                            [[900, 64], [50, pcnt], [1, 25]]),
                    in_=_ap(pt[:], sub * 64 * 512, [[512, 64], [25, pcnt], [1, 25]]),
                    func=AF.Copy, scale=1.0)
        sumT = work.tile([64, 36], F32)
        nc.vector.tensor_reduce(out=sumT[:], in_=expT[:], axis=mybir.AxisListType.X, op=OP.add)
        recT = work.tile([64, 36], F32)
        nc.vector.reciprocal(out=recT[:], in_=sumT[:])
        msm = work.tile([64, 36, 25], F32)
        nc.vector.tensor_tensor(out=msm[:], in0=expT[:],
                                in1=_ap(recT[:], 0, [[36, 64], [1, 36], [0, 25]]), op=OP.mult)
        # partition-shifted variants via SBUF-SBUF DMA (edges zero)
        msm_p1 = work.tile([64, 36, 25], F32)   # msm_p1[p] = msm[p+1]
        msm_m1 = work.tile([64, 36, 25], F32)   # msm_m1[p] = msm[p-1]
        nc.vector.memset(msm_p1[:], 0.0)
        nc.vector.memset(msm_m1[:], 0.0)
        nc.sync.dma_start(out=_ap(msm_p1[:], 0, [[900, 63], [1, 900]]),
                          in_=_ap(msm[:], 900, [[900, 63], [1, 900]]))
        nc.sync.dma_start(out=_ap(msm_m1[:], 900, [[900, 63], [1, 900]]),
                          in_=_ap(msm[:], 0, [[900, 63], [1, 900]]))

        # ---- 4. offset transpose; W9 ----
        deltT = work.tile([64, 32, 8], BF16)
        po_t = psc.tile([128, 512], F32, name='po_w', tag='pcs')
        po = po_t[:, 0:128]
        for i in range(16):
            nc.tensor.transpose(po_t[:, i * 8:i * 8 + 8],
                                _ap(offS[:], i * 128, [[32 * 64, 8], [1, 128]]),
                                id_sb[0:8, 0:8])
        for sub in range(2):
            nc.scalar.activation(
                out=_ap(deltT[:], sub * 8, [[256, 64], [16, 16], [1, 8]]),
                in_=_ap(po_t[:], sub * 64 * 512, [[512, 64], [8, 16], [1, 8]]),
                func=AF.Copy, scale=1.0)

        def dview(chbase):
            return _ap(deltT[:], chbase, [[256, 64], [8, 32], [1, 4]])

        def wt(nm):
            return work.tile([64, 128], BF16, name=nm)

        t1, t2 = wt('t1'), wt('t2')
        gxc, x0r, wxt, omwx, x1r = wt('gxc'), wt('x0r'), wt('wxt'), wt('omwx'), wt('x1r')
        gyc, y0r, wyt, omwy, y1r = wt('gyc'), wt('y0r'), wt('wyt'), wt('omwy'), wt('y1r')
        ia, ib = wt('ia'), wt('ib')
        cwx = work.tile([64, 3, 128], BF16)
        rwy = work.tile([64, 3, 128], BF16)
        W9b = work.tile([64, 9 * 128], BF16)


        hrow_bc = _ap(hrow_sb[:], 0, [[32, 64], [1, 32], [0, 4]])
        y63_bc = _ap(y63_sb[:], 0, [[32, 64], [1, 32], [0, 4]])

        def r4(ap):
            return _ap(ap, 0, [[128, 64], [4, 32], [1, 4]])

        nc.vector.tensor_scalar(out=t1[:], in0=dview(0), scalar1=wvec_sb[:], scalar2=None, op0=OP.add)
        nc.vector.tensor_scalar(out=t2[:], in0=t1[:], scalar1=0.0, scalar2=63.0, op0=OP.max, op1=OP.min)
        nc.vector.tensor_scalar(out=gxc[:], in0=t2[:], scalar1=wvec_sb[:], scalar2=None, op0=OP.subtract)
        nc.vector.tensor_scalar(out=x0r[:], in0=gxc[:], scalar1=0.0, scalar2=-1.0, op0=OP.is_lt, op1=OP.mult)
        nc.vector.tensor_tensor(out=wxt[:], in0=gxc[:], in1=x0r[:], op=OP.subtract)
        nc.vector.tensor_scalar(out=omwx[:], in0=wxt[:], scalar1=-1.0, scalar2=1.0, op0=OP.mult, op1=OP.add)
        nc.vector.tensor_scalar(out=x1r[:], in0=x0r[:], scalar1=1.0, scalar2=w63_sb[:], op0=OP.add, op1=OP.min)

        nc.vector.tensor_tensor(out=r4(t1[:]), in0=dview(4), in1=hrow_bc, op=OP.add)
        nc.vector.tensor_scalar(out=t2[:], in0=t1[:], scalar1=0.0, scalar2=63.0, op0=OP.max, op1=OP.min)
        nc.vector.tensor_tensor(out=r4(gyc[:]), in0=r4(t2[:]), in1=hrow_bc, op=OP.subtract)
        nc.vector.tensor_scalar(out=y0r[:], in0=gyc[:], scalar1=0.0, scalar2=-1.0, op0=OP.is_lt, op1=OP.mult)
        nc.vector.tensor_tensor(out=wyt[:], in0=gyc[:], in1=y0r[:], op=OP.subtract)
        nc.vector.tensor_scalar(out=omwy[:], in0=wyt[:], scalar1=-1.0, scalar2=1.0, op0=OP.mult, op1=OP.add)
        nc.vector.tensor_scalar(out=t1[:], in0=y0r[:], scalar1=1.0, scalar2=None, op0=OP.add)
        nc.vector.tensor_tensor(out=r4(y1r[:]), in0=r4(t1[:]), in1=y63_bc, op=OP.min)

        for i, e in enumerate((-1.0, 0.0, 1.0)):
            nc.vector.tensor_scalar(out=ia[:], in0=x0r[:], scalar1=e, scalar2=None, op0=OP.is_equal)
            nc.vector.tensor_scalar(out=ib[:], in0=x1r[:], scalar1=e, scalar2=None, op0=OP.is_equal)
            nc.vector.tensor_tensor(out=ia[:], in0=ia[:], in1=omwx[:], op=OP.mult)
            nc.vector.tensor_tensor(out=ib[:], in0=ib[:], in1=wxt[:], op=OP.mult)
            nc.vector.tensor_tensor(out=cwx[:, i, :], in0=ia[:], in1=ib[:], op=OP.add)
            nc.vector.tensor_scalar(out=ia[:], in0=y0r[:], scalar1=e, scalar2=None, op0=OP.is_equal)
            nc.vector.tensor_scalar(out=ib[:], in0=y1r[:], scalar1=e, scalar2=None, op0=OP.is_equal)
            nc.vector.tensor_tensor(out=ia[:], in0=ia[:], in1=omwy[:], op=OP.mult)
            nc.vector.tensor_tensor(out=ib[:], in0=ib[:], in1=wyt[:], op=OP.mult)
            nc.vector.tensor_tensor(out=rwy[:, i, :], in0=ia[:], in1=ib[:], op=OP.add)
        for iy in range(3):
            for ix in range(3):
                nc.vector.tensor_tensor(
                    out=_ap(W9b[:], (iy * 3 + ix) * 128, [[9 * 128, 64], [1, 128]]),
                    in0=rwy[:, iy, :], in1=cwx[:, ix, :], op=OP.mult)

        # ---- 5. kernc assembly, fp16, layout [64, (h:32, k:25, u:4)] ----
        msm4 = work.tile([64, 36, 25, 4], BF16)
        nc.vector.tensor_tensor(
            out=msm4[:],
            in0=_ap(expT[:], 0, [[900, 64], [25, 36], [1, 25], [0, 4]]),
            in1=_ap(recT[:], 0, [[36, 64], [1, 36], [0, 25], [0, 4]]), op=OP.mult)
        nc.sync.dma_start(out=_ap(msm4_p1[:], 0, [[3600, 63], [1, 3600]]),
                          in_=_ap(msm4[:], 3600, [[3600, 63], [1, 3600]]))
        nc.scalar.dma_start(out=_ap(msm4_m1[:], 3600, [[3600, 63], [1, 3600]]),
                          in_=_ap(msm4[:], 0, [[3600, 63], [1, 3600]]))
        kernc = work.tile([64, 3200], BF16)
        tmp = work.tile([64, 3200], BF16)
        msm_by_ex = {-1: msm4_m1, 0: msm4, 1: msm4_p1}
        kbf = {0: kernc}
        kbf.update(kbf_sh)
        data_all = work.tile([128, 16, 100], BF16)

        # ---- 5-9 software-pipelined by m-groups ----
        NG = 4
        GM = 16 // NG

        def emit_asm(G):
            for jh in range(2):
                hofs = (16 * jh + GM * G) * 100
                kv = _ap(kernc[:], hofs, [[3200, 64], [100, GM], [4, 25], [1, 4]])
                tv = _ap(tmp[:], hofs, [[3200, 64], [100, GM], [4, 25], [1, 4]])
                first = True
                for iy, ey in enumerate((-1, 0, 1)):
                    for ix, ex in enumerate((-1, 0, 1)):
                        mv = _ap(msm_by_ex[ex][:], (2 + ey + 16 * jh + GM * G) * 100,
                                 [[3600, 64], [100, GM], [4, 25], [1, 4]])
                        wv = _ap(W9b[:], (iy * 3 + ix) * 128 + (16 * jh + GM * G) * 4,
                                 [[9 * 128, 64], [4, GM], [0, 25], [1, 4]])
                        if first:
                            nc.vector.tensor_tensor(out=kv, in0=wv, in1=mv, op=OP.mult)
                            first = False
                        else:
                            nc.vector.tensor_tensor(out=tv, in0=wv, in1=mv, op=OP.mult)
                            nc.vector.tensor_tensor(out=kv, in0=kv, in1=tv, op=OP.add)

        def emit_dmas(G):
            for d in (-2, -1, 1, 2):
                cnt = 64 - abs(d)
                eng = nc.sync
                if d > 0:
                    eng.dma_start(
                        out=_ap(kbf[d][:], GM * G * 100, [[3200, cnt], [1600, 2], [1, GM * 100]]),
                        in_=_ap(kbf[0][:], d * 3200 + GM * G * 100,
                                [[3200, cnt], [1600, 2], [1, GM * 100]]))
                else:
                    eng.dma_start(
                        out=_ap(kbf[d][:], -d * 3200 + GM * G * 100,
                                [[3200, cnt], [1600, 2], [1, GM * 100]]),
                        in_=_ap(kbf[0][:], GM * G * 100, [[3200, cnt], [1600, 2], [1, GM * 100]]))

        def emit_prep(G):
            for jh in range(2):
                for b in range(5):
                    nc.vector.tensor_copy(
                        out=_ap(data_all[:], 64 * jh * 1600 + GM * G * 100 + b * 20,
                                [[1600, 64], [100, GM], [4, 5], [1, 4]]),
                        in_=_ap(kbf[b - 2][:], (16 * jh + GM * G) * 100 + (4 - b) * 4,
                                [[3200, 64], [100, GM], [20, 5], [1, 4]]))

        def emit_pairs(G):
            for m in range(GM * G, GM * G + GM):
                banded1 = band.tile([128, 1536], BF16, name=f'band1_{m}', tag='band1')
                banded2 = band.tile([128, 1024], BF16, name=f'band2_{m}', tag='band2')
                nc.gpsimd.local_scatter(out_ap=banded1[:], data_ap=data_all[:, m, :],
                                        idxs_ap=idx1_sb[:], channels=128, num_elems=1536, num_idxs=100)
                nc.gpsimd.local_scatter(out_ap=banded2[:], data_ap=data_all[:, m, :],
                                        idxs_ap=idx2_sb[:], channels=128, num_elems=1024, num_idxs=100)
                for ch in range(2):
                    pcs = psc.tile([128, 512], F32, name=f'pcs_{m}_{ch}', tag='pcs')
                    for ki in range(5):
                        lhsT = _ap(xT2_sb[:], (m + ki) * 256 + ch * 128, [[20 * 256, 128], [1, 128]])
                        rhs = banded1[:, ki * 512:ki * 512 + 512] if ki < 3 \
                            else banded2[:, (ki - 3) * 512:(ki - 3) * 512 + 512]
                        nc.tensor.matmul(pcs[:], lhsT, rhs, start=(ki == 0), stop=(ki == 4))
                    rb = rowp.tile([128, 512], F32, name=f'rb_{m}_{ch}', tag='rb')
                    nc.scalar.activation(out=rb[:], in_=pcs[:], func=AF.Copy, scale=1.0)
                    nc.scalar.dma_start(
                        out=_ap(outp[:], ch * 128 * 8192 + 4 * m * 128,
                                [[8192, 128], [128, 4], [1, 128]]),
                        in_=rb[:])

        emit_asm(0)
        emit_dmas(0)
        for G in range(NG):
            if G + 1 < NG:
                emit_asm(G + 1)
                emit_dmas(G + 1)
            emit_prep(G)
            emit_pairs(G)
    nc.finalize()
    return nc


_PROGRAM = None
_SCAT = build_scatter_tables()


def _get_program():
    global _PROGRAM
    if _PROGRAM is None:
        _PROGRAM = build_program()
    return _PROGRAM


def _prep_core_inputs(inputs, n, s):
    bf = np.float16
    x = np.asarray(inputs['x'][n], np.float32)
    h0 = 32 * s
    xw = np.zeros((C, 38, W), np.float32)
    for i, g in enumerate(range(h0 - 3, h0 + 35)):
        if 0 <= g < H:
            xw[:, i] = x[:, g]
    xwin = np.ascontiguousarray(xw.reshape(2, 128, 38 * 64)).astype(bf)
    xT2 = np.zeros((128, 20, C), np.float32)
    for jh in range(2):
        base = h0 + 16 * jh - 2
        for i in range(20):
            g = base + i
            if 0 <= g < H:
                xT2[64 * jh:64 * jh + 64, i] = x[:, g].T
    xT2 = np.ascontiguousarray(xT2.reshape(128, 20 * 256)).astype(bf)
    w_comp = np.asarray(inputs['w_comp'], np.float32)[:, :, 0, 0]
    wc = np.zeros((2, 128, 64), np.float32)
    for cg in range(2):
        wc[cg] = w_comp[:, cg * 128:(cg + 1) * 128].T
    wc = np.ascontiguousarray(wc.transpose(1, 0, 2).reshape(128, 2 * 64)).astype(bf)
    w_ker = np.asarray(inputs['w_ker'], np.float32)
    w_off = np.asarray(inputs['w_off'], np.float32)
    wk = np.zeros((9, 64, 57), np.float32)
    for t in range(9):
        wk[t, :, 0:8] = w_off[:, :, t // 3, t % 3].T
        wk[t, :, 32:57] = w_ker[:, :, t // 3, t % 3].T
    wk = np.ascontiguousarray(wk.transpose(1, 0, 2).reshape(64, 9 * 57)).astype(bf)
    bcov = np.zeros((57, 1), np.float32)
    bcov[0:8, 0] = np.asarray(inputs['b_off'], np.float32)
    bcov[32:57, 0] = np.asarray(inputs['b_ker'], np.float32)
    idx1, idx2 = _SCAT
    hr = (h0 + np.arange(32, dtype=np.float32))[None, :].repeat(64, 0)
    return {
        'xwin': xwin, 'xT2': xT2, 'wc': wc, 'wk': wk, 'bco': bcov,
        'bcomp': np.asarray(inputs['b_comp'], np.float32).reshape(64, 1),
        'wvec': np.arange(64, dtype=np.float32).reshape(64, 1),
        'w63': (63 - np.arange(64, dtype=np.float32)).reshape(64, 1),
        'hrow': np.ascontiguousarray(hr),
        'y63': np.ascontiguousarray(63.0 - hr),
        'ident': np.eye(128, dtype=np.float32),
        'idx1': idx1, 'idx2': idx2,
        'zed': np.zeros((2, 3600), np.float16),
    }


def kernel(**inputs):
    nc = _get_program()
    core_ids = list(range(8))
    in_maps = [_prep_core_inputs(inputs, cid // 2, cid % 2) for cid in core_ids]
    res = run_bass_kernel_spmd(nc, in_maps, core_ids)
    out = np.zeros((N, C, 128, 128), np.float32)
    for cid in core_ids:
        n, s = cid // 2, cid % 2
        op = np.asarray(res.results[cid]['outp']).reshape(256, 64, 128)
        out[n, :, s::2] = op
    return out


if __name__ == '__main__':
    d = np.load('/root/problem/ref_io.npz')
    inp = {k: d[k] for k in ('x', 'w_comp', 'b_comp', 'w_ker', 'b_ker', 'w_off', 'b_off')}
    out = kernel(**inp)
    ref = d['out']
    err = np.abs(out - ref).max()
    print('max abs err:', err, 'rel:', err / np.abs(ref).max())



# revision 3
# speedup vs baseline: 1.3237x; 1.3237x over previous
"""Trainium2 Bass kernel for nn_DLUPack (CARAFE-style dynamic upsampling), v2.

Sharding: 8 cores = (batch n in [0,4)) x (output-row-parity s in {0,1});
core (n, s) computes low-res rows hh in [32s, 32s+32) -> all parity-s output rows.

v2 layout: back phase jh-packed on 128 partitions, p = 64*jh + w.
  ref[n, c, 2y+i, 2x+j]: for core (n,s), y = h0 + 16*jh + m (h0=32s),
  out DRAM row r' = 4m + w//16, dcol = 8*(w%16) + 2u + jh, host: out[n,:,s::2].

Pipeline per core:
  1. compressor 1x1 conv (PE) -> cx [64, 38, 66] fp16
  2. offset+mask 3x3 convs (9 accumulated MMs x 6 groups) -> psum [57, 384]
  3. 16 po transposes -> deltT128 [128, 16, 8]; W9 indicator chain (DVE)
  4. 20 exp transposes -> expT128 [128, 20, 25] f32; softmax; msm4 [128,20,25,4] fp16
  5. +-1 w-shift variants of msm4 via SBUF-SBUF DMA (within 64-halves)
  6. kernc [128, 16m, 25k, 4u] assembly: 17 TT ops x 2 blocks (DVE, fp16 2x)
  7. kbf partition-shift variants (4 DMAs/blk); prep -> data_all [128, 16, 100]
  8. per m: local_scatter [128, 1280] (GPSIMD); per (jh, ch): 5 accumulated MMs
     lhsT=xT2[64jh.., 128c] rhs=banded[64jh.., ki*256..] -> psum [128c, 256px]
  9. ACT evac (fp16, jh-interleaved cols) -> rb group tile; 1 out DMA per (4m, ch)
"""
import sys
import numpy as np

sys.path.insert(0, '/opt/trn_rl_repo')

import ml_dtypes  # noqa: E402,F401
from contextlib import ExitStack  # noqa: E402

import concourse.bass as bass  # noqa: E402
import concourse.tile as tile  # noqa: E402
from concourse import mybir, bacc  # noqa: E402
from concourse.bass_utils import run_bass_kernel_spmd  # noqa: E402

F32 = mybir.dt.float32
FP16 = mybir.dt.float16
I16 = mybir.dt.int16
AF = mybir.ActivationFunctionType
OP = mybir.AluOpType

N, C, H, W = 4, 256, 64, 64
NWARM = 40


def _ap(base, off_elems, dims):
    return bass.AP(tensor=base.tensor, offset=base.offset + off_elems, ap=[list(d) for d in dims])


def build_scatter_table():
    # banded[p=64jh+pp, ki*256 + 4*w + u] = kernc[64jh + w, m, ki*5+(4-b), u],
    # w = pp + b - 2; data_all[p, (b*5+ki)*4+u] laid out by prep.
    idx = -np.ones((128, 100), np.int16)
    for p in range(128):
        pp = p % 64
        for b in range(5):
            w = pp + b - 2
            if not (0 <= w < 64):
                continue
            for ki in range(5):
                for u in range(4):
                    idx[p, (b * 5 + ki) * 4 + u] = ki * 256 + 4 * w + u
    return idx


# params [128, 40] f32 column map
P_WVEC, P_W63, P_HROW, P_Y63, P_BCOMP, P_BCO = 0, 1, 2, 18, 34, 35


def build_program():
    nc = bacc.Bacc(None, target_bir_lowering=False, debug=True)

    xwin = nc.declare_dram_parameter('xwin', [2, 128, 38 * 64], FP16, isOutput=False)
    xT2 = nc.declare_dram_parameter('xT2', [128, 20 * 256], FP16, isOutput=False)
    wc = nc.declare_dram_parameter('wc', [128, 2 * 64], FP16, isOutput=False)
    wk = nc.declare_dram_parameter('wk', [128, 6 * 57], FP16, isOutput=False)
    params = nc.declare_dram_parameter('params', [128, 40], F32, isOutput=False)
    ident = nc.declare_dram_parameter('ident', [128, 128], F32, isOutput=False)
    idxt = nc.declare_dram_parameter('idxt', [128, 100], I16, isOutput=False)
    zed = nc.declare_dram_parameter('zed', [2, 3600], FP16, isOutput=False)
    outp = nc.declare_dram_parameter('outp', [256, 64 * 128], FP16, isOutput=True)

    with tile.TileContext(nc) as tc, ExitStack() as ctx:
        sing = ctx.enter_context(tc.tile_pool(name='sing', bufs=1))
        work = ctx.enter_context(tc.tile_pool(name='work', bufs=1))
        band = ctx.enter_context(tc.tile_pool(name='band', bufs=4))
        rbp = ctx.enter_context(tc.tile_pool(name='rbp', bufs=2))
        psum = ctx.enter_context(tc.psum_pool(name='ps', bufs=2))
        tpp = ctx.enter_context(tc.psum_pool(name='tpp', bufs=2))
        psc = ctx.enter_context(tc.psum_pool(name='psc', bufs=4))

        def load(shape, dtype, src, name):
            t = sing.tile(shape, dtype, name=name)
            nc.sync.dma_start(out=t[:], in_=src[:])
            return t

        id_sb = load([128, 128], F32, ident, 'id')
        xwin_sb = sing.tile([128, 2, 38 * 64], FP16)
        for cg_ in range(2):
            nc.sync.dma_start(out=xwin_sb[:, cg_, :],
                              in_=_ap(xwin[:], cg_ * 128 * 2432, [[2432, 128], [1, 2432]]))
        xT2_sb = load([128, 20 * 256], FP16, xT2, 'xT2')
        wc_sb = load([128, 2, 64], FP16, wc, 'wc')
        wk_sb = load([128, 6 * 57], FP16, wk, 'wk')
        par_sb = load([128, 40], F32, params, 'par')
        idx_sb = load([128, 100], I16, idxt, 'idx')

        # PE warm-up while input DMAs land
        pw = psum.tile([128, 512], F32, name='warm', tag='front')
        for _ in range(NWARM):
            nc.tensor.matmul(pw[0:64, 0:64], id_sb[:, 0:64], id_sb[:, 0:64], start=True, stop=True)

        wvec = par_sb[:, P_WVEC:P_WVEC + 1]
        w63 = par_sb[:, P_W63:P_W63 + 1]
        bcomp = par_sb[0:64, P_BCOMP:P_BCOMP + 1]
        bker = _ap(par_sb[:], 32 * 40 + P_BCO, [[40, 25], [1, 1]])
        boff = par_sb[0:8, P_BCO:P_BCO + 1]
        hrow_bc = _ap(par_sb[:], P_HROW, [[40, 128], [1, 16], [0, 4]])
        y63_bc = _ap(par_sb[:], P_Y63, [[40, 128], [1, 16], [0, 4]])

        # hoisted shifted-variant buffers; edge partitions zeroed once (gpsimd)
        msm4 = work.tile([128, 20, 25, 4], FP16)
        msm4_p1 = work.tile([128, 20, 25, 4], FP16)   # [p] = msm4[p+1] within half
        msm4_m1 = work.tile([128, 20, 25, 4], FP16)   # [p] = msm4[p-1] within half
        for jh in range(2):
            nc.gpsimd.dma_start(
                out=_ap(msm4_p1[:], (jh * 64 + 63) * 2000, [[2000, 1], [1, 2000]]),
                in_=_ap(zed[:], 0, [[3600, 1], [1, 2000]]))
            nc.gpsimd.dma_start(
                out=_ap(msm4_m1[:], jh * 64 * 2000, [[2000, 1], [1, 2000]]),
                in_=_ap(zed[:], 0, [[3600, 1], [1, 2000]]))
        kernc = work.tile([128, 16 * 100], FP16)
        kbf = {0: kernc}
        for d in (-2, -1, 1, 2):
            kbf[d] = work.tile([128, 16 * 100], FP16, name=f'kbf{d}')
            for jh in range(2):
                if d > 0:
                    nc.gpsimd.dma_start(
                        out=_ap(kbf[d][:], (jh * 64 + 64 - d) * 1600, [[1600, d], [1, 1600]]),
                        in_=_ap(zed[:], 0, [[3600, d], [1, 1600]]))
                else:
                    nc.gpsimd.dma_start(
                        out=_ap(kbf[d][:], jh * 64 * 1600, [[1600, -d], [1, 1600]]),
                        in_=_ap(zed[:], 0, [[3600, -d], [1, 1600]]))

        # ---- 1. compressor ----
        # cx_sb [128, 38, 66]: lower half = cx rows; upper half = cx shifted
        # down one h-row (slot h holds row h+1) so taps (dy=0, dy=1) pack
        # into one 128-deep contraction.
        cx_sb = work.tile([128, 38, 66], FP16)
        nc.vector.memset(_ap(cx_sb[:], 0, [[38 * 66, 128], [66, 38], [1, 1]]), 0.0)
        nc.vector.memset(_ap(cx_sb[:], 65, [[38 * 66, 128], [66, 38], [1, 1]]), 0.0)
        for grp in range(5):
            g0 = grp * 8
            rows = min(8, 38 - g0)
            nn = rows * 64
            pcs = psum.tile([64, 512], F32, name=f'cmp{grp}', tag='front')
            for cg in range(2):
                nc.tensor.matmul(pcs[:, :nn], wc_sb[:, cg, :],
                                 xwin_sb[:, cg, g0 * 64:g0 * 64 + nn],
                                 start=(cg == 0), stop=(cg == 1))
            nc.scalar.activation(
                out=_ap(cx_sb[:], g0 * 66 + 1, [[38 * 66, 64], [66, rows], [1, 64]]),
                in_=_ap(pcs[:], 0, [[512, 64], [64, rows], [1, 64]]),
                func=AF.Identity, bias=bcomp, scale=1.0)
            r0 = max(g0, 1)
            cnt = (g0 + rows - r0) * 66
            nc.gpsimd.dma_start(
                out=_ap(cx_sb[:], 64 * 2508 + (r0 - 1) * 66, [[2508, 64], [1, cnt]]),
                in_=_ap(cx_sb[:], r0 * 66, [[2508, 64], [1, cnt]]))

        # ---- 2. offset+mask convs: 6 MMs (3 tap-pairs + 3 singles) ----
        # expS [25, t20, jh2, 64]: slot (t, jh) = conv row h = t + 16*jh
        # (h in [16,20) stored twice). offS [8, h'16, jh2, 64]: y = h' + 16*jh.
        expS = work.tile([25, 20, 2, 64], F32)
        offS = work.tile([8, 16, 2, 64], F32)
        for grp in range(6):
            g0 = grp * 6
            nn = 6 * 64
            pcs = psum.tile([57, 384], F32, name=f'off{grp}', tag='front')
            for s in range(6):
                if s < 3:  # pair: lower tap (0,s), upper tap (1,s)
                    lhsT = _ap(wk_sb[:], s * 57, [[6 * 57, 128], [1, 57]])
                    rhs = _ap(cx_sb[:], g0 * 66 + s, [[38 * 66, 128], [66, 6], [1, 64]])
                else:      # single: tap (2, s-3), lower half only
                    lhsT = _ap(wk_sb[:], s * 57, [[6 * 57, 64], [1, 57]])
                    rhs = _ap(cx_sb[:], (g0 + 2) * 66 + (s - 3),
                              [[38 * 66, 64], [66, 6], [1, 64]])
                nc.tensor.matmul(pcs[:, :nn], lhsT, rhs,
                                 start=(s == 0), stop=(s == 5))
            for jh in range(2):
                h_lo = max(g0, 20 * jh - 4)      # jh0: t=h in [0,20); jh1: t=h-16
                h_hi = min(g0 + 6, 20 + 16 * jh)
                if h_lo < h_hi:
                    nc.scalar.activation(
                        out=_ap(expS[:], (h_lo - 16 * jh) * 128 + jh * 64,
                                [[2560, 25], [128, h_hi - h_lo], [1, 64]]),
                        in_=_ap(pcs[:], 32 * 384 + (h_lo - g0) * 64,
                                [[384, 25], [64, h_hi - h_lo], [1, 64]]),
                        func=AF.Exp, bias=bker, scale=1.0)
                y_lo = max(g0 - 2, 16 * jh)
                y_hi = min(g0 + 4, 16 + 16 * jh)
                if y_lo < y_hi:
                    nc.vector.tensor_scalar(
                        out=_ap(offS[:], (y_lo - 16 * jh) * 128 + jh * 64,
                                [[2048, 8], [128, y_hi - y_lo], [1, 64]]),
                        in0=_ap(pcs[:], (y_lo + 2 - g0) * 64,
                                [[384, 8], [64, y_hi - y_lo], [1, 64]]),
                        scalar1=boff, scalar2=None, op0=OP.add)

        # ---- 3. offset transposes -> deltT128 [128, 16 h', 8 ch] ----
        po = tpp.tile([128, 512], F32, name='po', tag='tp')
        for hp in range(16):
            nc.tensor.transpose(po[:, hp * 8:hp * 8 + 8],
                                _ap(offS[:], hp * 128, [[2048, 8], [1, 128]]),
                                id_sb[0:8, 0:8])
        deltT = work.tile([128, 16, 8], FP16)
        nc.scalar.activation(out=deltT[:], in_=_ap(po[:], 0, [[512, 128], [1, 128]]),
                             func=AF.Copy, scale=1.0)

        # ---- 4. W9 indicator chain on [128, 64] ----
        def dview(chbase):
            return _ap(deltT[:], chbase, [[128, 128], [8, 16], [1, 4]])

        def wt(nm):
            return work.tile([128, 64], FP16, name=nm)

        t1, t2 = wt('t1'), wt('t2')
        gxc, x0r, wxt, omwx, x1r = wt('gxc'), wt('x0r'), wt('wxt'), wt('omwx'), wt('x1r')
        gyc, y0r, wyt, omwy, y1r = wt('gyc'), wt('y0r'), wt('wyt'), wt('omwy'), wt('y1r')
        ia, ib = wt('ia'), wt('ib')
        cwx = work.tile([128, 3, 64], FP16)
        rwy = work.tile([128, 3, 64], FP16)
        W9b = work.tile([128, 9, 64], FP16)

        def r4(ap):
            return _ap(ap, 0, [[64, 128], [4, 16], [1, 4]])

        nc.vector.tensor_scalar(out=t1[:], in0=dview(0), scalar1=wvec, scalar2=None, op0=OP.add)
        nc.vector.tensor_scalar(out=t2[:], in0=t1[:], scalar1=0.0, scalar2=63.0, op0=OP.max, op1=OP.min)
        nc.vector.tensor_scalar(out=gxc[:], in0=t2[:], scalar1=wvec, scalar2=None, op0=OP.subtract)
        nc.vector.tensor_scalar(out=x0r[:], in0=gxc[:], scalar1=0.0, scalar2=-1.0, op0=OP.is_lt, op1=OP.mult)
        nc.vector.tensor_tensor(out=wxt[:], in0=gxc[:], in1=x0r[:], op=OP.subtract)
        nc.vector.tensor_scalar(out=omwx[:], in0=wxt[:], scalar1=-1.0, scalar2=1.0, op0=OP.mult, op1=OP.add)
        nc.vector.tensor_scalar(out=x1r[:], in0=x0r[:], scalar1=1.0, scalar2=w63, op0=OP.add, op1=OP.min)

        nc.vector.tensor_tensor(out=r4(t1[:]), in0=dview(4), in1=hrow_bc, op=OP.add)
        nc.vector.tensor_scalar(out=t2[:], in0=t1[:], scalar1=0.0, scalar2=63.0, op0=OP.max, op1=OP.min)
        nc.vector.tensor_tensor(out=r4(gyc[:]), in0=r4(t2[:]), in1=hrow_bc, op=OP.subtract)
        nc.vector.tensor_scalar(out=y0r[:], in0=gyc[:], scalar1=0.0, scalar2=-1.0, op0=OP.is_lt, op1=OP.mult)
        nc.vector.tensor_tensor(out=wyt[:], in0=gyc[:], in1=y0r[:], op=OP.subtract)
        nc.vector.tensor_scalar(out=omwy[:], in0=wyt[:], scalar1=-1.0, scalar2=1.0, op0=OP.mult, op1=OP.add)
        nc.vector.tensor_scalar(out=t1[:], in0=y0r[:], scalar1=1.0, scalar2=None, op0=OP.add)
        nc.vector.tensor_tensor(out=r4(y1r[:]), in0=r4(t1[:]), in1=y63_bc, op=OP.min)

        for i, e in enumerate((-1.0, 0.0, 1.0)):
            nc.vector.tensor_scalar(out=ia[:], in0=x0r[:], scalar1=e, scalar2=None, op0=OP.is_equal)
            nc.vector.tensor_scalar(out=ib[:], in0=x1r[:], scalar1=e, scalar2=None, op0=OP.is_equal)
            nc.vector.tensor_tensor(out=ia[:], in0=ia[:], in1=omwx[:], op=OP.mult)
            nc.vector.tensor_tensor(out=ib[:], in0=ib[:], in1=wxt[:], op=OP.mult)
            nc.vector.tensor_tensor(out=cwx[:, i, :], in0=ia[:], in1=ib[:], op=OP.add)
            nc.vector.tensor_scalar(out=ia[:], in0=y0r[:], scalar1=e, scalar2=None, op0=OP.is_equal)
            nc.vector.tensor_scalar(out=ib[:], in0=y1r[:], scalar1=e, scalar2=None, op0=OP.is_equal)
            nc.vector.tensor_tensor(out=ia[:], in0=ia[:], in1=omwy[:], op=OP.mult)
            nc.vector.tensor_tensor(out=ib[:], in0=ib[:], in1=wyt[:], op=OP.mult)
            nc.vector.tensor_tensor(out=rwy[:, i, :], in0=ia[:], in1=ib[:], op=OP.add)
        for iy in range(3):
            for ix in range(3):
                nc.vector.tensor_tensor(
                    out=_ap(W9b[:], (iy * 3 + ix) * 64, [[9 * 64, 128], [1, 64]]),
                    in0=rwy[:, iy, :], in1=cwx[:, ix, :], op=OP.mult)

        # ---- 5. exp transposes -> expT128 [128, 20 t, 25 k]; softmax ----
        pt = tpp.tile([128, 512], F32, name='pt', tag='tp')
        for t in range(20):
            nc.tensor.transpose(pt[:, t * 25:t * 25 + 25],
                                _ap(expS[:], t * 128, [[2560, 25], [1, 128]]),
                                id_sb[0:25, 0:25])
        expT = work.tile([128, 20, 25], F32)
        nc.scalar.activation(out=expT[:], in_=_ap(pt[:], 0, [[512, 128], [1, 500]]),
                             func=AF.Copy, scale=1.0)
        sumT = work.tile([128, 20], F32)
        nc.vector.tensor_reduce(out=sumT[:], in_=expT[:], axis=mybir.AxisListType.X, op=OP.add)
        recT = work.tile([128, 20], F32)
        nc.vector.reciprocal(out=recT[:], in_=sumT[:])
        nc.vector.tensor_tensor(
            out=msm4[:],
            in0=_ap(expT[:], 0, [[500, 128], [25, 20], [1, 25], [0, 4]]),
            in1=_ap(recT[:], 0, [[20, 128], [1, 20], [0, 25], [0, 4]]), op=OP.mult)
        for jh in range(2):
            b0 = jh * 64 * 2000
            nc.sync.dma_start(out=_ap(msm4_p1[:], b0, [[2000, 63], [1, 2000]]),
                              in_=_ap(msm4[:], b0 + 2000, [[2000, 63], [1, 2000]]))
            nc.sync.dma_start(out=_ap(msm4_m1[:], b0 + 2000, [[2000, 63], [1, 2000]]),
                              in_=_ap(msm4[:], b0, [[2000, 63], [1, 2000]]))

        # ---- 6-9. kernc assembly + banded + carafe, 2 blocks of 8 m ----
        msm_by_ex = {-1: msm4_m1, 0: msm4, 1: msm4_p1}
        data_all = work.tile([128, 16, 100], FP16)

        def emit_asm(blk):
            hofs = blk * 8 * 100
            kv = _ap(kernc[:], hofs, [[1600, 128], [100, 8], [4, 25], [1, 4]])
            tv = _ap(tmp_asm[:], 0, [[800, 128], [100, 8], [4, 25], [1, 4]])
            first = True
            for iy, ey in enumerate((-1, 0, 1)):
                for ix, ex in enumerate((-1, 0, 1)):
                    mv = _ap(msm_by_ex[ex][:], (2 + ey + blk * 8) * 100,
                             [[2000, 128], [100, 8], [4, 25], [1, 4]])
                    wv = _ap(W9b[:], (iy * 3 + ix) * 64 + blk * 8 * 4,
                             [[9 * 64, 128], [4, 8], [0, 25], [1, 4]])
                    if first:
                        nc.vector.tensor_tensor(out=kv, in0=wv, in1=mv, op=OP.mult)
                        first = False
                    else:
                        nc.vector.tensor_tensor(out=tv, in0=wv, in1=mv, op=OP.mult)
                        nc.vector.tensor_tensor(out=kv, in0=kv, in1=tv, op=OP.add)

        tmp_asm = work.tile([128, 800], FP16)

        def emit_kbf(blk):
            for i, d in enumerate((-2, -1, 1, 2)):
                eng = nc.sync if i % 2 == 0 else nc.scalar
                for jh in range(2):
                    b0 = jh * 64 * 1600 + blk * 800
                    if d > 0:
                        eng.dma_start(
                            out=_ap(kbf[d][:], b0, [[1600, 64 - d], [1, 800]]),
                            in_=_ap(kernc[:], b0 + d * 1600, [[1600, 64 - d], [1, 800]]))
                    else:
                        eng.dma_start(
                            out=_ap(kbf[d][:], b0 - d * 1600, [[1600, 64 + d], [1, 800]]),
                            in_=_ap(kernc[:], b0, [[1600, 64 + d], [1, 800]]))

        def emit_prep(blk):
            for b in range(5):
                nc.vector.tensor_copy(
                    out=_ap(data_all[:], blk * 800 + b * 20,
                            [[1600, 128], [100, 8], [4, 5], [1, 4]]),
                    in_=_ap(kbf[b - 2][:], blk * 800 + (4 - b) * 4,
                            [[1600, 128], [100, 8], [20, 5], [1, 4]]))

        rb_t = [None, None]

        def emit_m(m):
            banded = band.tile([128, 1280], FP16, name=f'band_{m}', tag='band')
            nc.gpsimd.local_scatter(out_ap=banded[:], data_ap=data_all[:, m, :],
                                    idxs_ap=idx_sb[:], channels=128, num_elems=1280,
                                    num_idxs=100)
            if m % 4 == 0:
                g = m // 4
                for ch in range(2):
                    rb_t[ch] = rbp.tile([128, 4 * 512], FP16, name=f'rb_{g}_{ch}', tag=f'rb{ch}')
            for jh in range(2):
                for ch in range(2):
                    pcs = psc.tile([128, 256], F32, name=f'pcs_{m}_{jh}_{ch}', tag='pcs')
                    for ki in range(5):
                        lhsT = _ap(xT2_sb[:], jh * 64 * 5120 + (m + ki) * 256 + ch * 128,
                                   [[5120, 64], [1, 128]])
                        rhs = _ap(banded[:], jh * 64 * 1280 + ki * 256, [[1280, 64], [1, 256]])
                        nc.tensor.matmul(pcs[:], lhsT, rhs, start=(ki == 0), stop=(ki == 4))
                    nc.scalar.activation(
                        out=_ap(rb_t[ch][:], (m % 4) * 512 + jh,
                                [[4 * 512, 128], [128, 4], [8, 16], [2, 4]]),
                        in_=_ap(pcs[:], 0, [[256, 128], [64, 4], [4, 16], [1, 4]]),
                        func=AF.Copy, scale=1.0)
            if m % 4 == 3:
                g = m // 4
                for ch in range(2):
                    nc.scalar.dma_start(
                        out=_ap(outp[:], ch * 128 * 8192 + 16 * g * 128,
                                [[8192, 128], [128, 16], [1, 128]]),
                        in_=rb_t[ch][:])

        for blk in range(2):
            emit_asm(blk)
            emit_kbf(blk)
            emit_prep(blk)
            for m in range(blk * 8, blk * 8 + 8):
                emit_m(m)
    nc.finalize()
    return nc


_PROGRAM = None
_SCAT = build_scatter_table()


def _get_program():
    global _PROGRAM
    if _PROGRAM is None:
        _PROGRAM = build_program()
    return _PROGRAM


def _prep_core_inputs(inputs, n, s):
    bf = np.float16
    x = np.asarray(inputs['x'][n], np.float32)
    h0 = 32 * s
    xw = np.zeros((C, 38, W), np.float32)
    for i, g in enumerate(range(h0 - 3, h0 + 35)):
        if 0 <= g < H:
            xw[:, i] = x[:, g]
    xwin = np.ascontiguousarray(xw.reshape(2, 128, 38 * 64)).astype(bf)
    xT2 = np.zeros((128, 20, C), np.float32)
    for jh in range(2):
        base = h0 + 16 * jh - 2
        for i in range(20):
            g = base + i
            if 0 <= g < H:
                xT2[64 * jh:64 * jh + 64, i] = x[:, g].T
    xT2 = np.ascontiguousarray(xT2.reshape(128, 20 * 256)).astype(bf)
    w_comp = np.asarray(inputs['w_comp'], np.float32)[:, :, 0, 0]
    wc = np.zeros((2, 128, 64), np.float32)
    for cg in range(2):
        wc[cg] = w_comp[:, cg * 128:(cg + 1) * 128].T
    wc = np.ascontiguousarray(wc.transpose(1, 0, 2).reshape(128, 2 * 64)).astype(bf)
    w_ker = np.asarray(inputs['w_ker'], np.float32)
    w_off = np.asarray(inputs['w_off'], np.float32)
    wkT = np.zeros((9, 64, 57), np.float32)
    for t in range(9):
        wkT[t, :, 0:8] = w_off[:, :, t // 3, t % 3].T
        wkT[t, :, 32:57] = w_ker[:, :, t // 3, t % 3].T
    wk = np.zeros((128, 6, 57), np.float32)
    for s, t in enumerate((0, 1, 2, 6, 7, 8)):
        wk[0:64, s] = wkT[t]
    for s, t in enumerate((3, 4, 5)):
        wk[64:128, s] = wkT[t]
    wk = np.ascontiguousarray(wk.reshape(128, 6 * 57)).astype(bf)

    par = np.zeros((128, 40), np.float32)
    p = np.arange(128)
    wv = (p % 64).astype(np.float32)
    jh = (p // 64).astype(np.float32)
    par[:, P_WVEC] = wv
    par[:, P_W63] = 63.0 - wv
    hh = h0 + 16.0 * jh[:, None] + np.arange(16, dtype=np.float32)[None, :]
    par[:, P_HROW:P_HROW + 16] = hh
    par[:, P_Y63:P_Y63 + 16] = 63.0 - hh
    par[0:64, P_BCOMP] = np.asarray(inputs['b_comp'], np.float32)
    bcov = np.zeros(128, np.float32)
    bcov[0:8] = np.asarray(inputs['b_off'], np.float32)
    bcov[32:57] = np.asarray(inputs['b_ker'], np.float32)
    par[:, P_BCO] = bcov

    return {
        'xwin': xwin, 'xT2': xT2, 'wc': wc, 'wk': wk, 'params': par,
        'ident': np.eye(128, dtype=np.float32),
        'idxt': _SCAT,
        'zed': np.zeros((2, 3600), np.float16),
    }


def kernel(**inputs):
    nc = _get_program()
    core_ids = list(range(8))
    in_maps = [_prep_core_inputs(inputs, cid // 2, cid % 2) for cid in core_ids]
    res = run_bass_kernel_spmd(nc, in_maps, core_ids)
    out = np.zeros((N, C, 128, 128), np.float32)
    for cid in core_ids:
        n, s = cid // 2, cid % 2
        op = np.asarray(res.results[cid]['outp']).astype(np.float32).reshape(256, 64, 128)
        out[n, :, s::2] = op
    return out


if __name__ == '__main__':
    d = np.load('/root/problem/ref_io.npz')
    inp = {k: d[k] for k in ('x', 'w_comp', 'b_comp', 'w_ker', 'b_ker', 'w_off', 'b_off')}
    out = kernel(**inp)
    ref = d['out']
    err = np.abs(out - ref).max()
    print('max abs err:', err, 'rel:', err / np.abs(ref).max())


# revision 4
# speedup vs baseline: 1.4487x; 1.0944x over previous
"""Trainium2 Bass kernel for nn_DLUPack (CARAFE-style dynamic upsampling), v2.

Sharding: 8 cores = (batch n in [0,4)) x (output-row-parity s in {0,1});
core (n, s) computes low-res rows hh in [32s, 32s+32) -> all parity-s output rows.

v2 layout: back phase jh-packed on 128 partitions, p = 64*jh + w.
  ref[n, c, 2y+i, 2x+j]: for core (n,s), y = h0 + 16*jh + m (h0=32s),
  out DRAM row r' = 4m + w//16, dcol = 8*(w%16) + 2u + jh, host: out[n,:,s::2].

Pipeline per core:
  1. compressor 1x1 conv (PE) -> cx [64, 38, 66] fp16
  2. offset+mask 3x3 convs (9 accumulated MMs x 6 groups) -> psum [57, 384]
  3. 16 po transposes -> deltT128 [128, 16, 8]; W9 indicator chain (DVE)
  4. 20 exp transposes -> expT128 [128, 20, 25] f32; softmax; msm4 [128,20,25,4] fp16
  5. +-1 w-shift variants of msm4 via SBUF-SBUF DMA (within 64-halves)
  6. kernc [128, 16m, 25k, 4u] assembly: 17 TT ops x 2 blocks (DVE, fp16 2x)
  7. kbf partition-shift variants (4 DMAs/blk); prep -> data_all [128, 16, 100]
  8. per m: local_scatter [128, 1280] (GPSIMD); per (jh, ch): 5 accumulated MMs
     lhsT=xT2[64jh.., 128c] rhs=banded[64jh.., ki*256..] -> psum [128c, 256px]
  9. ACT evac (fp16, jh-interleaved cols) -> rb group tile; 1 out DMA per (4m, ch)
"""
import sys
import numpy as np

sys.path.insert(0, '/opt/trn_rl_repo')

import ml_dtypes  # noqa: E402,F401
from contextlib import ExitStack  # noqa: E402

import concourse.bass as bass  # noqa: E402
import concourse.tile as tile  # noqa: E402
from concourse import mybir, bacc  # noqa: E402
from concourse.bass_utils import run_bass_kernel_spmd  # noqa: E402

F32 = mybir.dt.float32
FP16 = mybir.dt.float16
I16 = mybir.dt.int16
AF = mybir.ActivationFunctionType
OP = mybir.AluOpType

N, C, H, W = 4, 256, 64, 64
NWARM = 24


def _ap(base, off_elems, dims):
    return bass.AP(tensor=base.tensor, offset=base.offset + off_elems, ap=[list(d) for d in dims])


def build_scatter_table():
    # banded[p=64jh+pp, ki*256 + 4*w + u] = kernc[64jh + w, m, ki*5+(4-b), u],
    # w = pp + b - 2; data_all[p, (b*5+ki)*4+u] laid out by prep.
    idx = -np.ones((128, 100), np.int16)
    for p in range(128):
        pp = p % 64
        for b in range(5):
            w = pp + b - 2
            if not (0 <= w < 64):
                continue
            for ki in range(5):
                for u in range(4):
                    idx[p, (b * 5 + ki) * 4 + u] = ki * 256 + 4 * w + u
    return idx


# params [128, 40] f32 column map
P_WVEC, P_W63, P_HROW, P_Y63, P_BCOMP, P_BCO = 0, 1, 2, 18, 34, 35


def build_program():
    nc = bacc.Bacc(None, target_bir_lowering=False, debug=True)

    xwin = nc.declare_dram_parameter('xwin', [2, 128, 38 * 64], FP16, isOutput=False)
    xT2 = nc.declare_dram_parameter('xT2', [128, 20 * 256], FP16, isOutput=False)
    wc = nc.declare_dram_parameter('wc', [128, 2 * 64], FP16, isOutput=False)
    wk = nc.declare_dram_parameter('wk', [128, 6 * 57], FP16, isOutput=False)
    params = nc.declare_dram_parameter('params', [128, 40], F32, isOutput=False)
    ident = nc.declare_dram_parameter('ident', [128, 128], F32, isOutput=False)
    idxt = nc.declare_dram_parameter('idxt', [128, 100], I16, isOutput=False)
    zed = nc.declare_dram_parameter('zed', [2, 3600], FP16, isOutput=False)
    outp = nc.declare_dram_parameter('outp', [256, 64 * 128], FP16, isOutput=True)

    with tile.TileContext(nc) as tc, ExitStack() as ctx:
        sing = ctx.enter_context(tc.tile_pool(name='sing', bufs=1))
        work = ctx.enter_context(tc.tile_pool(name='work', bufs=1))
        band = ctx.enter_context(tc.tile_pool(name='band', bufs=4))
        rbp = ctx.enter_context(tc.tile_pool(name='rbp', bufs=2))
        psum = ctx.enter_context(tc.psum_pool(name='ps', bufs=2))
        psc = ctx.enter_context(tc.psum_pool(name='psc', bufs=6))

        def load(shape, dtype, src, name):
            t = sing.tile(shape, dtype, name=name)
            nc.sync.dma_start(out=t[:], in_=src[:])
            return t

        id_sb = load([128, 128], F32, ident, 'id')
        xwin_sb = sing.tile([128, 2, 38 * 64], FP16)
        for cg_ in range(2):
            nc.sync.dma_start(out=xwin_sb[:, cg_, :],
                              in_=_ap(xwin[:], cg_ * 128 * 2432, [[2432, 128], [1, 2432]]))
        xT2_sb = load([128, 20 * 256], FP16, xT2, 'xT2')
        wc_sb = load([128, 2, 64], FP16, wc, 'wc')
        wk_sb = load([128, 6 * 57], FP16, wk, 'wk')
        par_sb = load([128, 40], F32, params, 'par')
        idx_sb = load([128, 100], I16, idxt, 'idx')

        # PE warm-up while input DMAs land; dummy ACT pulls the table load early
        pw = psum.tile([128, 512], F32, name='warm', tag='front')
        dumt = work.tile([1, 4], F32, name='dumt')
        nc.scalar.activation(out=dumt[:], in_=id_sb[0:1, 0:4], func=AF.Copy, scale=1.0)
        for _ in range(NWARM):
            nc.tensor.matmul(pw[0:64, 0:64], id_sb[:, 0:64], id_sb[:, 0:64], start=True, stop=True)

        wvec = par_sb[:, P_WVEC:P_WVEC + 1]
        w63 = par_sb[:, P_W63:P_W63 + 1]
        bcomp = par_sb[0:64, P_BCOMP:P_BCOMP + 1]
        bker = _ap(par_sb[:], 32 * 40 + P_BCO, [[40, 25], [1, 1]])
        boff = par_sb[0:8, P_BCO:P_BCO + 1]
        hrow_bc = _ap(par_sb[:], P_HROW, [[40, 128], [1, 16], [0, 4]])
        y63_bc = _ap(par_sb[:], P_Y63, [[40, 128], [1, 16], [0, 4]])

        # hoisted shifted-variant buffers; edge partitions zeroed once (gpsimd)
        msm4 = work.tile([128, 20, 25, 4], FP16)
        msm4_p1 = work.tile([128, 20, 25, 4], FP16)   # [p] = msm4[p+1] within half
        msm4_m1 = work.tile([128, 20, 25, 4], FP16)   # [p] = msm4[p-1] within half
        for jh in range(2):
            nc.gpsimd.dma_start(
                out=_ap(msm4_p1[:], (jh * 64 + 63) * 2000, [[2000, 1], [1, 2000]]),
                in_=_ap(zed[:], 0, [[3600, 1], [1, 2000]]))
            nc.gpsimd.dma_start(
                out=_ap(msm4_m1[:], jh * 64 * 2000, [[2000, 1], [1, 2000]]),
                in_=_ap(zed[:], 0, [[3600, 1], [1, 2000]]))
        kernc = work.tile([128, 16 * 100], FP16)
        kbf = {0: kernc}
        for d in (-2, -1, 1, 2):
            kbf[d] = work.tile([128, 16 * 100], FP16, name=f'kbf{d}')
            for jh in range(2):
                if d > 0:
                    nc.gpsimd.dma_start(
                        out=_ap(kbf[d][:], (jh * 64 + 64 - d) * 1600, [[1600, d], [1, 1600]]),
                        in_=_ap(zed[:], 0, [[3600, d], [1, 1600]]))
                else:
                    nc.gpsimd.dma_start(
                        out=_ap(kbf[d][:], jh * 64 * 1600, [[1600, -d], [1, 1600]]),
                        in_=_ap(zed[:], 0, [[3600, -d], [1, 1600]]))

        # ---- 1. compressor ----
        # cx_sb [128, 38, 66]: lower half = cx rows; upper half = cx shifted
        # down one h-row (slot h holds row h+1) so taps (dy=0, dy=1) pack
        # into one 128-deep contraction.
        cx_sb = work.tile([128, 38, 66], FP16)
        nc.vector.memset(_ap(cx_sb[:], 0, [[38 * 66, 128], [66, 38], [1, 1]]), 0.0)
        nc.vector.memset(_ap(cx_sb[:], 65, [[38 * 66, 128], [66, 38], [1, 1]]), 0.0)
        for grp in range(5):
            g0 = grp * 8
            rows = min(8, 38 - g0)
            nn = rows * 64
            pcs = psum.tile([64, 512], F32, name=f'cmp{grp}', tag='front')
            for cg in range(2):
                nc.tensor.matmul(pcs[:, :nn], wc_sb[:, cg, :],
                                 xwin_sb[:, cg, g0 * 64:g0 * 64 + nn],
                                 start=(cg == 0), stop=(cg == 1))
            nc.scalar.activation(
                out=_ap(cx_sb[:], g0 * 66 + 1, [[38 * 66, 64], [66, rows], [1, 64]]),
                in_=_ap(pcs[:], 0, [[512, 64], [64, rows], [1, 64]]),
                func=AF.Identity, bias=bcomp, scale=1.0)
            r0 = max(g0, 1)
            cnt = (g0 + rows - r0) * 66
            nc.gpsimd.dma_start(
                out=_ap(cx_sb[:], 64 * 2508 + (r0 - 1) * 66, [[2508, 64], [1, cnt]]),
                in_=_ap(cx_sb[:], r0 * 66, [[2508, 64], [1, cnt]]))

        # ---- 2. offset+mask convs: 6 MMs (3 tap-pairs + 3 singles) ----
        # expS [25, t20, jh2, 64]: slot (t, jh) = conv row h = t + 16*jh
        # (h in [16,20) stored twice). offS [8, h'16, jh2, 64]: y = h' + 16*jh.
        expS = work.tile([25, 20, 2, 64], F32)
        offS = work.tile([8, 16, 2, 64], F32)
        for grp in range(6):
            g0 = grp * 6
            nn = 6 * 64
            pcs = psum.tile([57, 384], F32, name=f'off{grp}', tag='front')
            for s in range(6):
                if s < 3:  # pair: lower tap (0,s), upper tap (1,s)
                    lhsT = _ap(wk_sb[:], s * 57, [[6 * 57, 128], [1, 57]])
                    rhs = _ap(cx_sb[:], g0 * 66 + s, [[38 * 66, 128], [66, 6], [1, 64]])
                else:      # single: tap (2, s-3), lower half only
                    lhsT = _ap(wk_sb[:], s * 57, [[6 * 57, 64], [1, 57]])
                    rhs = _ap(cx_sb[:], (g0 + 2) * 66 + (s - 3),
                              [[38 * 66, 64], [66, 6], [1, 64]])
                nc.tensor.matmul(pcs[:, :nn], lhsT, rhs,
                                 start=(s == 0), stop=(s == 5))
            for jh in range(2):
                h_lo = max(g0, 20 * jh - 4)      # jh0: t=h in [0,20); jh1: t=h-16
                h_hi = min(g0 + 6, 20 + 16 * jh)
                if h_lo < h_hi:
                    nc.scalar.activation(
                        out=_ap(expS[:], (h_lo - 16 * jh) * 128 + jh * 64,
                                [[2560, 25], [128, h_hi - h_lo], [1, 64]]),
                        in_=_ap(pcs[:], 32 * 384 + (h_lo - g0) * 64,
                                [[384, 25], [64, h_hi - h_lo], [1, 64]]),
                        func=AF.Exp, bias=bker, scale=1.0)
                y_lo = max(g0 - 2, 16 * jh)
                y_hi = min(g0 + 4, 16 + 16 * jh)
                if y_lo < y_hi:
                    nc.vector.tensor_scalar(
                        out=_ap(offS[:], (y_lo - 16 * jh) * 128 + jh * 64,
                                [[2048, 8], [128, y_hi - y_lo], [1, 64]]),
                        in0=_ap(pcs[:], (y_lo + 2 - g0) * 64,
                                [[384, 8], [64, y_hi - y_lo], [1, 64]]),
                        scalar1=boff, scalar2=None, op0=OP.add)

        # ---- 3. offset transposes -> deltT128 [128, 16 h', 8 ch] ----
        po = psum.tile([128, 512], F32, name='po', tag='front')
        for hp in range(16):
            nc.tensor.transpose(po[:, hp * 8:hp * 8 + 8],
                                _ap(offS[:], hp * 128, [[2048, 8], [1, 128]]),
                                id_sb[0:8, 0:8])
        deltT = work.tile([128, 16, 8], FP16)
        nc.scalar.activation(out=deltT[:], in_=_ap(po[:], 0, [[512, 128], [1, 128]]),
                             func=AF.Copy, scale=1.0)

        # ---- 4. W9 indicator chain on [128, 64] ----
        def dview(chbase):
            return _ap(deltT[:], chbase, [[128, 128], [8, 16], [1, 4]])

        def wt(nm):
            return work.tile([128, 64], FP16, name=nm)

        t1, t2 = wt('t1'), wt('t2')
        gxc, x0r, wxt, omwx, x1r = wt('gxc'), wt('x0r'), wt('wxt'), wt('omwx'), wt('x1r')
        gyc, y0r, wyt, omwy, y1r = wt('gyc'), wt('y0r'), wt('wyt'), wt('omwy'), wt('y1r')
        ia, ib = wt('ia'), wt('ib')
        cwx = work.tile([128, 3, 64], FP16)
        rwy = work.tile([128, 3, 64], FP16)
        W9b = work.tile([128, 9, 64], FP16)

        def r4(ap):
            return _ap(ap, 0, [[64, 128], [4, 16], [1, 4]])

        nc.vector.tensor_scalar(out=t1[:], in0=dview(0), scalar1=wvec, scalar2=None, op0=OP.add)
        nc.vector.tensor_scalar(out=t2[:], in0=t1[:], scalar1=0.0, scalar2=63.0, op0=OP.max, op1=OP.min)
        nc.vector.tensor_scalar(out=gxc[:], in0=t2[:], scalar1=wvec, scalar2=None, op0=OP.subtract)
        nc.vector.tensor_scalar(out=x0r[:], in0=gxc[:], scalar1=0.0, scalar2=-1.0, op0=OP.is_lt, op1=OP.mult)
        nc.vector.tensor_tensor(out=wxt[:], in0=gxc[:], in1=x0r[:], op=OP.subtract)
        nc.vector.tensor_scalar(out=omwx[:], in0=wxt[:], scalar1=-1.0, scalar2=1.0, op0=OP.mult, op1=OP.add)
        nc.vector.tensor_scalar(out=x1r[:], in0=x0r[:], scalar1=1.0, scalar2=w63, op0=OP.add, op1=OP.min)

        nc.vector.tensor_tensor(out=r4(t1[:]), in0=dview(4), in1=hrow_bc, op=OP.add)
        nc.vector.tensor_scalar(out=t2[:], in0=t1[:], scalar1=0.0, scalar2=63.0, op0=OP.max, op1=OP.min)
        nc.vector.tensor_tensor(out=r4(gyc[:]), in0=r4(t2[:]), in1=hrow_bc, op=OP.subtract)
        nc.vector.tensor_scalar(out=y0r[:], in0=gyc[:], scalar1=0.0, scalar2=-1.0, op0=OP.is_lt, op1=OP.mult)
        nc.vector.tensor_tensor(out=wyt[:], in0=gyc[:], in1=y0r[:], op=OP.subtract)
        nc.vector.tensor_scalar(out=omwy[:], in0=wyt[:], scalar1=-1.0, scalar2=1.0, op0=OP.mult, op1=OP.add)
        nc.vector.tensor_scalar(out=t1[:], in0=y0r[:], scalar1=1.0, scalar2=None, op0=OP.add)
        nc.vector.tensor_tensor(out=r4(y1r[:]), in0=r4(t1[:]), in1=y63_bc, op=OP.min)

        for i, e in enumerate((-1.0, 0.0, 1.0)):
            nc.vector.tensor_scalar(out=ia[:], in0=x0r[:], scalar1=e, scalar2=None, op0=OP.is_equal)
            nc.vector.tensor_scalar(out=ib[:], in0=x1r[:], scalar1=e, scalar2=None, op0=OP.is_equal)
            nc.vector.tensor_tensor(out=ia[:], in0=ia[:], in1=omwx[:], op=OP.mult)
            nc.vector.tensor_tensor(out=ib[:], in0=ib[:], in1=wxt[:], op=OP.mult)
            nc.vector.tensor_tensor(out=cwx[:, i, :], in0=ia[:], in1=ib[:], op=OP.add)
            nc.vector.tensor_scalar(out=ia[:], in0=y0r[:], scalar1=e, scalar2=None, op0=OP.is_equal)
            nc.vector.tensor_scalar(out=ib[:], in0=y1r[:], scalar1=e, scalar2=None, op0=OP.is_equal)
            nc.vector.tensor_tensor(out=ia[:], in0=ia[:], in1=omwy[:], op=OP.mult)
            nc.vector.tensor_tensor(out=ib[:], in0=ib[:], in1=wyt[:], op=OP.mult)
            nc.vector.tensor_tensor(out=rwy[:, i, :], in0=ia[:], in1=ib[:], op=OP.add)
        for iy in range(3):
            for ix in range(3):
                nc.vector.tensor_tensor(
                    out=_ap(W9b[:], (iy * 3 + ix) * 64, [[9 * 64, 128], [1, 64]]),
                    in0=rwy[:, iy, :], in1=cwx[:, ix, :], op=OP.mult)

        # ---- 5. exp transposes -> expT128 [128, 20 t, 25 k]; softmax ----
        pt = psum.tile([128, 512], F32, name='pt', tag='front')
        for t in range(20):
            nc.tensor.transpose(pt[:, t * 25:t * 25 + 25],
                                _ap(expS[:], t * 128, [[2560, 25], [1, 128]]),
                                id_sb[0:25, 0:25])
        expT = work.tile([128, 20, 25], F32)
        nc.scalar.activation(out=expT[:], in_=_ap(pt[:], 0, [[512, 128], [1, 500]]),
                             func=AF.Copy, scale=1.0)
        sumT = work.tile([128, 20], F32)
        nc.vector.tensor_reduce(out=sumT[:], in_=expT[:], axis=mybir.AxisListType.X, op=OP.add)
        recT = work.tile([128, 20], F32)
        nc.vector.reciprocal(out=recT[:], in_=sumT[:])
        nc.vector.tensor_tensor(
            out=msm4[:],
            in0=_ap(expT[:], 0, [[500, 128], [25, 20], [1, 25], [0, 4]]),
            in1=_ap(recT[:], 0, [[20, 128], [1, 20], [0, 25], [0, 4]]), op=OP.mult)
        shift_engs = (nc.gpsimd, nc.sync, nc.scalar, nc.gpsimd)
        for jh in range(2):
            b0 = jh * 64 * 2000
            shift_engs[2 * jh].dma_start(
                out=_ap(msm4_p1[:], b0, [[2000, 63], [1, 2000]]),
                in_=_ap(msm4[:], b0 + 2000, [[2000, 63], [1, 2000]]))
            shift_engs[2 * jh + 1].dma_start(
                out=_ap(msm4_m1[:], b0 + 2000, [[2000, 63], [1, 2000]]),
                in_=_ap(msm4[:], b0, [[2000, 63], [1, 2000]]))

        # ---- 6-9. kernc assembly + banded + carafe, 4 blocks of 4 m ----
        msm_by_ex = {-1: msm4_m1, 0: msm4, 1: msm4_p1}
        data_all = work.tile([128, 16, 100], FP16)
        pbuf = [work.tile([128, 400], FP16, name=f'pb{t}') for t in range(9)]

        def emit_asm(blk):
            # 9 independent products, then a pairwise reduction tree
            GM = 4
            kv = _ap(kernc[:], blk * GM * 100, [[1600, 128], [100, GM], [4, 25], [1, 4]])

            def pv(t):
                return _ap(pbuf[t][:], 0, [[400, 128], [100, GM], [4, 25], [1, 4]])

            for t, (ey, ex) in enumerate((ey, ex) for ey in (-1, 0, 1) for ex in (-1, 0, 1)):
                mv = _ap(msm_by_ex[ex][:], (2 + ey + blk * GM) * 100,
                         [[2000, 128], [100, GM], [4, 25], [1, 4]])
                wv = _ap(W9b[:], t * 64 + blk * GM * 4,
                         [[9 * 64, 128], [4, GM], [0, 25], [1, 4]])
                nc.vector.tensor_tensor(out=pv(t), in0=wv, in1=mv, op=OP.mult)
            for a, b in ((0, 1), (2, 3), (4, 5), (6, 7), (0, 2), (4, 6), (0, 4)):
                nc.vector.tensor_tensor(out=pv(a), in0=pv(a), in1=pv(b), op=OP.add)
            nc.vector.tensor_tensor(out=kv, in0=pv(0), in1=pv(8), op=OP.add)

        def emit_kbf(blk):
            engs = (nc.sync, nc.scalar, nc.gpsimd, nc.gpsimd)
            for i, d in enumerate((-2, -1, 1, 2)):
                eng = engs[i]
                for jh in range(2):
                    b0 = jh * 64 * 1600 + blk * 400
                    if d > 0:
                        eng.dma_start(
                            out=_ap(kbf[d][:], b0, [[1600, 64 - d], [1, 400]]),
                            in_=_ap(kernc[:], b0 + d * 1600, [[1600, 64 - d], [1, 400]]))
                    else:
                        eng.dma_start(
                            out=_ap(kbf[d][:], b0 - d * 1600, [[1600, 64 + d], [1, 400]]),
                            in_=_ap(kernc[:], b0, [[1600, 64 + d], [1, 400]]))

        def emit_prep(blk):
            for b in range(5):
                nc.vector.tensor_copy(
                    out=_ap(data_all[:], blk * 400 + b * 20,
                            [[1600, 128], [100, 4], [4, 5], [1, 4]]),
                    in_=_ap(kbf[b - 2][:], blk * 400 + (4 - b) * 4,
                            [[1600, 128], [100, 4], [20, 5], [1, 4]]))

        rb_t = [None, None]

        def emit_m(m):
            banded = band.tile([128, 1280], FP16, name=f'band_{m}', tag='band')
            nc.gpsimd.local_scatter(out_ap=banded[:], data_ap=data_all[:, m, :],
                                    idxs_ap=idx_sb[:], channels=128, num_elems=1280,
                                    num_idxs=100)
            if m % 4 == 0:
                g = m // 4
                for ch in range(2):
                    rb_t[ch] = rbp.tile([128, 4 * 512], FP16, name=f'rb_{g}_{ch}', tag=f'rb{ch}')
            for jh in range(2):
                for ch in range(2):
                    pcs = psc.tile([128, 256], F32, name=f'pcs_{m}_{jh}_{ch}', tag='pcs')
                    for ki in range(5):
                        lhsT = _ap(xT2_sb[:], jh * 64 * 5120 + (m + ki) * 256 + ch * 128,
                                   [[5120, 64], [1, 128]])
                        rhs = _ap(banded[:], jh * 64 * 1280 + ki * 256, [[1280, 64], [1, 256]])
                        nc.tensor.matmul(pcs[:], lhsT, rhs, start=(ki == 0), stop=(ki == 4))
                    nc.scalar.activation(
                        out=_ap(rb_t[ch][:], (m % 4) * 512 + jh,
                                [[4 * 512, 128], [128, 4], [8, 16], [2, 4]]),
                        in_=_ap(pcs[:], 0, [[256, 128], [64, 4], [4, 16], [1, 4]]),
                        func=AF.Copy, scale=1.0)
            if m % 4 == 3:
                g = m // 4
                for ch in range(2):
                    nc.sync.dma_start(
                        out=_ap(outp[:], ch * 128 * 8192 + 16 * g * 128,
                                [[8192, 128], [128, 16], [1, 128]]),
                        in_=rb_t[ch][:])

        for blk in range(4):
            emit_asm(blk)
            emit_kbf(blk)
            emit_prep(blk)
            for m in range(blk * 4, blk * 4 + 4):
                emit_m(m)
    nc.finalize()
    return nc


_PROGRAM = None
_SCAT = build_scatter_table()


def _get_program():
    global _PROGRAM
    if _PROGRAM is None:
        _PROGRAM = build_program()
    return _PROGRAM


def _prep_core_inputs(inputs, n, s):
    bf = np.float16
    x = np.asarray(inputs['x'][n], np.float32)
    h0 = 32 * s
    xw = np.zeros((C, 38, W), np.float32)
    for i, g in enumerate(range(h0 - 3, h0 + 35)):
        if 0 <= g < H:
            xw[:, i] = x[:, g]
    xwin = np.ascontiguousarray(xw.reshape(2, 128, 38 * 64)).astype(bf)
    xT2 = np.zeros((128, 20, C), np.float32)
    for jh in range(2):
        base = h0 + 16 * jh - 2
        for i in range(20):
            g = base + i
            if 0 <= g < H:
                xT2[64 * jh:64 * jh + 64, i] = x[:, g].T
    xT2 = np.ascontiguousarray(xT2.reshape(128, 20 * 256)).astype(bf)
    w_comp = np.asarray(inputs['w_comp'], np.float32)[:, :, 0, 0]
    wc = np.zeros((2, 128, 64), np.float32)
    for cg in range(2):
        wc[cg] = w_comp[:, cg * 128:(cg + 1) * 128].T
    wc = np.ascontiguousarray(wc.transpose(1, 0, 2).reshape(128, 2 * 64)).astype(bf)
    w_ker = np.asarray(inputs['w_ker'], np.float32)
    w_off = np.asarray(inputs['w_off'], np.float32)
    wkT = np.zeros((9, 64, 57), np.float32)
    for t in range(9):
        wkT[t, :, 0:8] = w_off[:, :, t // 3, t % 3].T
        wkT[t, :, 32:57] = w_ker[:, :, t // 3, t % 3].T
    wk = np.zeros((128, 6, 57), np.float32)
    for s, t in enumerate((0, 1, 2, 6, 7, 8)):
        wk[0:64, s] = wkT[t]
    for s, t in enumerate((3, 4, 5)):
        wk[64:128, s] = wkT[t]
    wk = np.ascontiguousarray(wk.reshape(128, 6 * 57)).astype(bf)

    par = np.zeros((128, 40), np.float32)
    p = np.arange(128)
    wv = (p % 64).astype(np.float32)
    jh = (p // 64).astype(np.float32)
    par[:, P_WVEC] = wv
    par[:, P_W63] = 63.0 - wv
    hh = h0 + 16.0 * jh[:, None] + np.arange(16, dtype=np.float32)[None, :]
    par[:, P_HROW:P_HROW + 16] = hh
    par[:, P_Y63:P_Y63 + 16] = 63.0 - hh
    par[0:64, P_BCOMP] = np.asarray(inputs['b_comp'], np.float32)
    bcov = np.zeros(128, np.float32)
    bcov[0:8] = np.asarray(inputs['b_off'], np.float32)
    bcov[32:57] = np.asarray(inputs['b_ker'], np.float32)
    par[:, P_BCO] = bcov

    return {
        'xwin': xwin, 'xT2': xT2, 'wc': wc, 'wk': wk, 'params': par,
        'ident': np.eye(128, dtype=np.float32),
        'idxt': _SCAT,
        'zed': np.zeros((2, 3600), np.float16),
    }


def kernel(**inputs):
    nc = _get_program()
    core_ids = list(range(8))
    in_maps = [_prep_core_inputs(inputs, cid // 2, cid % 2) for cid in core_ids]
    res = run_bass_kernel_spmd(nc, in_maps, core_ids)
    out = np.zeros((N, C, 128, 128), np.float32)
    for cid in core_ids:
        n, s = cid // 2, cid % 2
        op = np.asarray(res.results[cid]['outp']).astype(np.float32).reshape(256, 64, 128)
        out[n, :, s::2] = op
    return out


if __name__ == '__main__':
    d = np.load('/root/problem/ref_io.npz')
    inp = {k: d[k] for k in ('x', 'w_comp', 'b_comp', 'w_ker', 'b_ker', 'w_off', 'b_off')}
    out = kernel(**inp)
    ref = d['out']
    err = np.abs(out - ref).max()
    print('max abs err:', err, 'rel:', err / np.abs(ref).max())
